# revision 17
# baseline (speedup 1.0000x reference)
"""Region-augmented embedding lookup (MeanEncoder) on 8 TRN2 NeuronCores.

Reference computation (per batch b, position l):
    out[b,l,0,:] = tanh( sum_{j=0..6} W[ seq_pad[b, l+j]*7 + j , :] ) * (seq[b,l]!=0)

Strategy: data parallel, W replicated (cast to bf16 on host), each core
takes 2 of 16 sequences.

Device kernel, super-groups of 4 tiles (each tile = 122 output positions
from 128 gathered window positions):
  1. Four per-tile indirect DMAs gather per-token contiguous 7x128 bf16
     blocks W[tok*7 : tok*7+7, :] into a [128, 4*896] SBUF tile (the TRN2
     indirect DMA consumes exactly one index per dest partition and
     streams 1792B from that base address). Gathers are round-robined
     over 4 SWDGE queues so descriptor generation can overlap.
  2. Shifted region-sum out[i] = sum_j G[i+j, seg_j] on the tensor
     engine: 7 bf16 matmuls, lhsT = identity slice ID[:, j:j+122]
     (stationary), rhs = the j-th 128-col segment of all 4 tiles
     (N=512 moving), PSUM fp32-accumulated (exact given bf16 inputs).
  3. One scalar-engine activation tanh(psum) per group writing bf16,
     then one batched store DMA per group (488 rows), alternating the
     sync/scalar HWDGE queues.
The (seq!=0) mask and the fp32 upcast are applied on the host during
unshard. Out-of-sequence window positions use token id 0 (= the
reference pad). bf16 keeps max rel err ~4e-3, inside the 2e-2 gate.
"""

import numpy as np
import ml_dtypes

import concourse.bass as bass
import concourse.tile as tile
from concourse import bacc, mybir
from concourse.bass_utils import run_bass_kernel_spmd

VOCAB = 50000
EMB = 128
RADIUS = 3
REGION = 7
B, L, C = 16, 2048, 1
NCORES = 8
SEQ_PER_CORE = B // NCORES           # 2
P = 128                              # gathered window positions per tile
TOUT = P - (REGION - 1)              # 122 output positions per tile
TILES_PER_SEQ = -(-L // TOUT)        # 17 (16*122=1952, last tile 96 rows)
NTILES = SEQ_PER_CORE * TILES_PER_SEQ  # 34
GRP = 4                              # tiles per super-group
BLK = REGION * EMB                   # 896
def _build_nc():
    nc = bacc.Bacc("TRN2", target_bir_lowering=False, debug=False)

    w = nc.declare_dram_parameter("w", [VOCAB * REGION, EMB], mybir.dt.bfloat16, isOutput=False)
    gidx = nc.declare_dram_parameter("gidx", [P, NTILES], mybir.dt.int32, isOutput=False)
    ident = nc.declare_dram_parameter("ident", [P, P], mybir.dt.bfloat16, isOutput=False)
    out = nc.declare_dram_parameter("out", [SEQ_PER_CORE * L, EMB], mybir.dt.bfloat16, isOutput=True)

    # tile t -> (sequence s, within-seq k); groups of 4 consecutive tiles
    # within one sequence, the 17th tile of each sequence stands alone.
    # per sequence: 4+4+4+2+2 full tiles -- the trailing groups are small so
    # the final full-group matmul chain in the kernel tail is short.
    full_groups = []
    for s in range(SEQ_PER_CORE):
        base = s * TILES_PER_SEQ
        full_groups += [[base + k0 + u for u in range(n)]
                        for k0, n in ((0, 4), (4, 4), (8, 4), (12, 2), (14, 2))]
    # the two ragged 96-row tiles form ONE combined group issued LAST: the
    # final full group's heavy store then overlaps the gather phase, and the
    # kernel tail is a single small group (one matmul chain, one store).
    ragged = [TILES_PER_SEQ - 1, 2 * TILES_PER_SEQ - 1]
    groups = full_groups + [ragged]

    from contextlib import ExitStack
    with tile.TileContext(nc) as tc, ExitStack() as ctx:
        const_pool = ctx.enter_context(tc.tile_pool(name="const", bufs=1))
        gpool = ctx.enter_context(tc.tile_pool(name="gather", bufs=8))
        ppool = ctx.enter_context(tc.tile_pool(name="psum", bufs=8, space="PSUM"))
        opool = ctx.enter_context(tc.tile_pool(name="out", bufs=8))

        gidx_sb = const_pool.tile([P, NTILES], mybir.dt.int32)
        id_sb = const_pool.tile([P, P], mybir.dt.bfloat16)
        # gidx gates the gather chain -- load it from gpsimd itself (the
        # HWDGE engines' preamble runs ~2us longer than gpsimd's, so a
        # sync/scalar-issued load would delay the first gather);
        # ident rides the scalar-engine HWDGE in parallel.
        nc.gpsimd.dma_start(gidx_sb[:], gidx.ap())
        nc.scalar.dma_start(id_sb[:], ident.ap())

        store_engines = [nc.sync, nc.scalar]
        for gi, g_tiles in enumerate(groups):
            ng = len(g_tiles)
            is_ragged = g_tiles == ragged
            t0 = g_tiles[0]
            s, k0 = divmod(t0, TILES_PER_SEQ)
            row0 = s * L + k0 * TOUT
            nrows_last = min(TOUT, L - (k0 + ng - 1) * TOUT)
            tot_rows = (ng - 1) * TOUT + nrows_last

            gsb = gpool.tile([P, GRP * BLK], mybir.dt.bfloat16, tag="g")
            for u, t in enumerate(g_tiles):
                nc.gpsimd.indirect_dma_start(
                    out=gsb[:, u * BLK:(u + 1) * BLK],
                    out_offset=None,
                    in_=w.ap(),
                    in_offset=bass.IndirectOffsetOnAxis(ap=gidx_sb[:, t: t + 1], axis=0),
                )
            psum = ppool.tile([TOUT, GRP * EMB], mybir.dt.float32, tag="ps")
            # rhs for offset j: the j-th 128-col segment of each tile's block
            gv = gsb[:].rearrange("p (u j e) -> p u j e", u=GRP, j=REGION)
            for j in range(REGION):
                nc.tensor.matmul(
                    out=psum[:, : ng * EMB],
                    lhsT=id_sb[:, j: j + TOUT],
                    rhs=gv[:, :ng, j, :],
                    start=(j == 0),
                    stop=(j == REGION - 1),
                )
            o = opool.tile([TOUT, GRP * EMB], mybir.dt.bfloat16, tag="o")
            nc.scalar.activation(
                o[:, : ng * EMB], psum[:, : ng * EMB],
                mybir.ActivationFunctionType.Tanh,
            )
            # one store per group, alternating HWDGE queues:
            # SBUF (i, u, e) -> DRAM rows row0 + u*TOUT + i
            eng = store_engines[gi % 2]
            if is_ragged:
                # tiles (16, 33): rows [1952:2048) of each sequence half
                nr = L - (TILES_PER_SEQ - 1) * TOUT          # 96
                dst = out.ap().rearrange("(u l) e -> u l e", u=2)[:, L - nr:, :] \
                    .rearrange("u i e -> i u e")
                src = o[:].rearrange("i (u e) -> i u e", u=GRP)[:nr, :2, :]
                eng.dma_start(dst, src)
            elif ng == 1:
                eng.dma_start(out.ap()[row0: row0 + tot_rows, :], o[:tot_rows, :EMB])
            else:
                dst = out.ap()[row0: row0 + ng * TOUT, :].rearrange("(u i) e -> i u e", u=ng)
                src = o[:].rearrange("i (u e) -> i u e", u=GRP)[:, :ng, :]
                eng.dma_start(dst, src)
    nc.compile()
    return nc


def _host_prep(seq, W):
    s = seq.reshape(B, L)
    ident = np.eye(P, dtype=ml_dtypes.bfloat16)
    w16 = np.ascontiguousarray(W.astype(ml_dtypes.bfloat16))

    in_maps = []
    for c in range(NCORES):
        gidx_r = np.zeros((P, NTILES), np.int32)
        for t in range(NTILES):
            sq, k = divmod(t, TILES_PER_SEQ)
            b = c * SEQ_PER_CORE + sq
            q0 = k * TOUT
            v = q0 - RADIUS + np.arange(P)
            tok = np.where((v >= 0) & (v < L), s[b, np.clip(v, 0, L - 1)], 0)
            gidx_r[:, t] = tok.astype(np.int32) * REGION
        in_maps.append({
            "w": w16,
            "gidx": gidx_r,
            "ident": ident,
        })
    return in_maps


_NC_CACHE = None


def run(seq, W, trace=False, **spmd_kwargs):
    global _NC_CACHE
    if _NC_CACHE is None:
        _NC_CACHE = _build_nc()
    nc = _NC_CACHE
    seq = np.asarray(seq)
    in_maps = _host_prep(seq, W)
    res = run_bass_kernel_spmd(
        nc, in_maps, core_ids=list(range(NCORES)), trace=trace, **spmd_kwargs
    )
    outs = [r["out"] for r in res.results]                 # each [2*L, EMB] bf16
    full = np.stack(outs, axis=0).reshape(B, L, EMB).astype(np.float32)
    full *= (seq.reshape(B, L, 1) != 0)
    return full[:, :, None, :], res


def kernel(seq, W):
    out, _ = run(np.asarray(seq), np.asarray(W))
    return out
